# revision 1
# baseline (speedup 1.0000x reference)
"""GAT (8-layer, 8-head) Trainium2 Bass kernel, 8-core SPMD.

Strategy:
- Host: add self-loops, sort edges by dst, partition dst nodes into 8 equal
  node-range shards (20 windows of 128 dst nodes per core), pad each
  (core, window) edge list to a uniform TPW*128 slots.
- Device, per layer: each core computes, for ITS node shard, a fused
  [h | s] = x @ [W | W@A2] (PE, fp32), transposes to node-major 256B table
  rows [h bf16 (cols 0:64) | s_src f32 (f32-cols 32:40) | s_dst f32 (40:48)],
  AllGather -> full 20481-row table in local HBM.
  Per window: dma_gather full rows by src and by dst (<=1024 idx per call,
  a Q7 ucode limit), e = leakyrelu(s_src + s_dst), ex = exp(e) (softmax
  WITHOUT max subtraction: exact in exact arithmetic, safe since |e| << 80),
  R = [ex*h | ex] bf16, one-hot S (dst_local == iota) bf16 built on DVE,
  scatter-reduce via PE matmul psum[j,72] += S^T R accumulated over the
  window's edge tiles, then out[j] = psum[j,:64] / (psum[j,64:72]+1e-16) + b.
- Padding edge slots point at a sentinel table row with s_src = -1e30
  (=> ex = 0) and dst_local = -1 (=> all-zero one-hot column).
"""

import os
import numpy as np
import ml_dtypes

N_NODES = 20000
N_EDGES = 640000
L, H, C = 8, 8, 8
D = H * C  # 64
NEG_SLOPE = 0.2

NCORES = 8
WIN = 128                 # dst nodes per window
WPC = 20                  # windows per core
NSH = WIN * WPC           # 2560 nodes per shard
NPAD = NCORES * NSH       # 20480
SENT = NPAD               # sentinel node id (table row)
TROWS = NPAD + 1          # table rows (incl. sentinel)

_cache = {}
REPEAT = 1
ABLATE = set()  # {"B","GATH","S","ER","MM","EVAC"}


# ----------------------------------------------------------------------------
# Host preprocessing
# ----------------------------------------------------------------------------
def _prep_edges(edge_index):
    src = np.asarray(edge_index[0], dtype=np.int64)
    dst = np.asarray(edge_index[1], dtype=np.int64)
    src = np.concatenate([src, np.arange(N_NODES, dtype=np.int64)])
    dst = np.concatenate([dst, np.arange(N_NODES, dtype=np.int64)])
    order = np.argsort(dst, kind="stable")
    src, dst = src[order], dst[order]

    nwin = NCORES * WPC  # 160
    win_of_edge = dst // WIN
    counts = np.bincount(win_of_edge, minlength=nwin)
    tpw = int(np.ceil(counts.max() / 128))
    nsw = tpw * 128                      # slots per window
    nslot = WPC * nsw                    # slots per core

    # slot arrays per core
    src_slot = np.full((NCORES, nslot), SENT, dtype=np.int64)
    dst_slot = np.full((NCORES, nslot), SENT, dtype=np.int64)
    dloc_slot = np.full((NCORES, nslot), -1.0, dtype=np.float32)

    wstart = np.zeros(nwin + 1, dtype=np.int64)
    np.cumsum(counts, out=wstart[1:])
    for w in range(nwin):
        c, wl = divmod(w, WPC)
        e0, e1 = wstart[w], wstart[w + 1]
        s0 = wl * nsw
        n = e1 - e0
        src_slot[c, s0:s0 + n] = src[e0:e1]
        dst_slot[c, s0:s0 + n] = dst[e0:e1]
        dloc_slot[c, s0:s0 + n] = (dst[e0:e1] - w * WIN).astype(np.float32)

    def wrap16(a):
        # index i -> [16*rep + i%16, i//16] for rep 0..7
        w = a.reshape(-1, 16).T.astype(np.int16)      # [16, nslot/16]
        return np.tile(w, (8, 1)).copy()              # [128, nslot/16]

    def wrap128(a):
        return a.reshape(-1, 128).T.copy()            # [128, nslot/128]

    srcidx = np.stack([wrap16(src_slot[c]) for c in range(NCORES)])
    dstidx = np.stack([wrap16(dst_slot[c]) for c in range(NCORES)])
    dloc = np.stack([wrap128(dloc_slot[c]) for c in range(NCORES)]).astype(
        ml_dtypes.bfloat16)
    return tpw, srcidx, dstidx, dloc


# ----------------------------------------------------------------------------
# Bass program
# ----------------------------------------------------------------------------
def _build(tpw):
    import concourse.bass as bass
    import concourse.tile as tile
    import concourse.mybir as mybir
    from concourse import bacc
    from contextlib import ExitStack

    f32 = mybir.dt.float32
    bf16 = mybir.dt.bfloat16
    i16 = mybir.dt.int16
    Alu = mybir.AluOpType
    Act = mybir.ActivationFunctionType

    nsw = tpw * 128
    nslot = WPC * nsw

    nc = bacc.Bacc("TRN2", target_bir_lowering=False, debug=False,
                   num_devices=NCORES)

    # external I/O
    t_xsh = nc.dram_tensor("xsh", [NSH, D], f32, kind="ExternalInput")
    t_srci = nc.dram_tensor("srcidx", [128, nslot // 16], i16, kind="ExternalInput")
    t_dsti = nc.dram_tensor("dstidx", [128, nslot // 16], i16, kind="ExternalInput")
    t_dloc = nc.dram_tensor("dstloc", [128, nslot // 128], bf16, kind="ExternalInput")
    t_iota = nc.dram_tensor("iota", [128, 128], bf16, kind="ExternalInput")
    t_ident = nc.dram_tensor("ident", [128, 128], f32, kind="ExternalInput")
    t_wts = nc.dram_tensor("wts", [64, L, 80], f32, kind="ExternalInput")
    t_brep = nc.dram_tensor("brep", [128, L, 64], f32, kind="ExternalInput")
    t_out = nc.dram_tensor("out", [NSH, D], f32, kind="ExternalOutput")

    with tile.TileContext(nc) as tc, ExitStack() as ctx:
        cpool = ctx.enter_context(tc.tile_pool(name="const", bufs=1))
        wpool = ctx.enter_context(tc.tile_pool(name="work", bufs=2))
        gpool = ctx.enter_context(tc.tile_pool(name="gath", bufs=2))
        epool = ctx.enter_context(tc.tile_pool(name="edge", bufs=3))
        dram = ctx.enter_context(tc.tile_pool(name="dram", bufs=1, space="DRAM"))
        psA = ctx.enter_context(tc.tile_pool(name="psA", bufs=2, space="PSUM"))
        psT = ctx.enter_context(tc.tile_pool(name="psT", bufs=2, space="PSUM"))
        psW = ctx.enter_context(tc.tile_pool(name="psW", bufs=2, space="PSUM"))

        # persistent SBUF
        sb_x = cpool.tile([128, WPC, D], f32)          # node-major shard x
        sb_srci = cpool.tile([128, nslot // 16], i16)
        sb_dsti = cpool.tile([128, nslot // 16], i16)
        sb_dloc = cpool.tile([128, nslot // 128], bf16)
        sb_iota = cpool.tile([128, 128], bf16)
        sb_ident = cpool.tile([128, 128], f32)
        sb_wts = cpool.tile([64, L, 80], f32)
        sb_brep = cpool.tile([128, L, 64], f32)

        nc.sync.dma_start(sb_x[:], t_xsh.ap().rearrange("(t p) c -> p t c", p=128))
        nc.sync.dma_start(sb_srci[:], t_srci.ap())
        nc.sync.dma_start(sb_dsti[:], t_dsti.ap())
        nc.sync.dma_start(sb_dloc[:], t_dloc.ap())
        nc.sync.dma_start(sb_iota[:], t_iota.ap())
        nc.sync.dma_start(sb_ident[:], t_ident.ap())
        nc.sync.dma_start(sb_wts[:], t_wts.ap())
        nc.sync.dma_start(sb_brep[:], t_brep.ap())

        # DRAM: gather table + staging shard. bf16 rows (256B):
        # [0:64] h bf16; f32 view: [32:40] s_src, [40:48] s_dst, [48:64] pad
        TAB = dram.tile([TROWS, 128], bf16)
        STAGE = dram.tile([NSH, 128], bf16)

        # STAGE bf16 cols 96:128 (f32 48:64) are never produced; zero once
        zjunk = cpool.tile([128, WPC, 32], bf16)
        nc.vector.memset(zjunk[:], 0.0)
        nc.sync.dma_start(
            STAGE[:, 96:128].rearrange("(t p) c -> p t c", p=128), zjunk[:])

        # sentinel row: h=0, s_src=-1e30 (=> ex = 0 for padding), s_dst=0
        sent = cpool.tile([1, 128], bf16)
        nc.vector.memset(sent[:], 0.0)
        nc.vector.memset(sent[:].bitcast(f32)[:, 32:40], -1e30)
        nc.sync.dma_start(TAB[SENT:SENT + 1, :], sent[:])

        for rep_l in range(REPEAT * L):
            l = rep_l % L
            # ---------------- phase A: per-node prep (own shard) ----------
            xT = wpool.tile([64, NSH], f32, tag="xT")
            for t in range(WPC):
                pt = psT.tile([64, 128], f32)
                nc.tensor.transpose(pt[:], sb_x[:, t, :], sb_ident[:])
                nc.scalar.copy(xT[:, t * 128:(t + 1) * 128], pt[:])

            # hs_T = [W | W@A2]^T @ x^T : [80, NSH] = [h_T ; s_T]
            hsT = wpool.tile([80, NSH], f32, tag="hsT")
            for k0 in range(0, NSH, 512):
                k1 = min(k0 + 512, NSH)
                ph = psA.tile([80, k1 - k0], f32, tag="psA")
                nc.tensor.matmul(ph[:], lhsT=sb_wts[:, l, :],
                                 rhs=xT[:, k0:k1], start=True, stop=True)
                nc.scalar.copy(hsT[:, k0:k1], ph[:])

            # node-major table rows: transpose [80, 128] -> [128, 80]
            tabsb = wpool.tile([128, WPC, 128], bf16, tag="tabsb")
            for t in range(WPC):
                pt = psT.tile([128, 80], f32, tag="psTb")
                nc.tensor.transpose(pt[:], hsT[:, t * 128:(t + 1) * 128],
                                    sb_ident[:80, :80])
                nc.scalar.copy(tabsb[:, t, 0:64], pt[:, 0:64])
                nc.vector.tensor_copy(
                    tabsb[:, t, :].bitcast(f32)[:, 32:48], pt[:, 64:80])

            nc.sync.dma_start(
                STAGE[:, 0:96].rearrange("(t p) c -> p t c", p=128),
                tabsb[:, :, 0:96])
            nc.gpsimd.collective_compute(
                "AllGather", Alu.bypass,
                replica_groups=[list(range(NCORES))],
                ins=[STAGE[:].opt()],
                outs=[TAB[0:NPAD, :].opt()],
            )

            # ---------------- phase B: edges, per window ------------------
            for w in range(WPC if "B" not in ABLATE else 0):
                # dma_gather is limited to 1024 indices per call
                GCH = 8
                vs = gpool.tile([128, tpw, 128], bf16, tag="vsrc")
                vd = gpool.tile([128, tpw, 128], bf16, tag="vdst")
                for t0 in (range(0, tpw, GCH) if "GATH" not in ABLATE else []):
                    t1 = min(t0 + GCH, tpw)
                    n = (t1 - t0) * 128
                    i0 = (w * nsw + t0 * 128) // 16
                    i1 = (w * nsw + t1 * 128) // 16
                    nc.gpsimd.dma_gather(
                        out_ap=vs[:, t0:t1, :], in_ap=TAB[:],
                        idxs_ap=sb_srci[:, i0:i1],
                        num_idxs=n, num_idxs_reg=n, elem_size=128)
                    nc.gpsimd.dma_gather(
                        out_ap=vd[:, t0:t1, :], in_ap=TAB[:],
                        idxs_ap=sb_dsti[:, i0:i1],
                        num_idxs=n, num_idxs_reg=n, elem_size=128)

                # one-hot S: [128, tpw*128] bf16
                S = epool.tile([128, tpw, 128], bf16, tag="S")
                dl = sb_dloc[:, w * tpw:(w + 1) * tpw]
                if "S" in ABLATE:
                    nc.vector.memset(S[:], 0.0)
                else:
                    nc.vector.tensor_tensor(
                        S[:],
                        dl.unsqueeze(2).broadcast_to([128, tpw, 128]),
                        sb_iota[:].unsqueeze(1).broadcast_to([128, tpw, 128]),
                        Alu.is_equal)

                # e = lrelu(s_src + s_dst); ex = exp(e)
                ex = epool.tile([128, tpw, 8], f32, tag="ex")
                R = epool.tile([128, tpw, 72], bf16, tag="R")
                if "ER" in ABLATE:
                    nc.vector.memset(ex[:], 0.5)
                    nc.vector.memset(R[:], 0.5)
                else:
                    e = epool.tile([128, tpw, 8], f32, tag="e")
                    nc.vector.tensor_tensor(
                        e[:], vs[:].bitcast(f32)[:, :, 32:40],
                        vd[:].bitcast(f32)[:, :, 40:48], Alu.add)
                    nc.vector.scalar_tensor_tensor(e[:], e[:], NEG_SLOPE, e[:],
                                                   op0=Alu.mult, op1=Alu.max)
                    nc.scalar.activation(ex[:], e[:], Act.Exp)
                    # R = [V*ex | ex] in bf16
                    nc.vector.tensor_copy(R[:, :, 64:72], ex[:])
                    nc.vector.tensor_tensor(
                        R[:, :, 0:64].rearrange("p t (h c) -> p t h c", h=8),
                        vs[:, :, 0:64].rearrange("p t (h c) -> p t h c", h=8),
                        R[:, :, 64:72].unsqueeze(3).broadcast_to(
                            [128, tpw, 8, 8]),
                        Alu.mult)

                pw = psW.tile([128, 72], f32)
                if "MM" in ABLATE:
                    nc.vector.memset(pw[:], 1.0)
                else:
                    for t in range(tpw):
                        nc.tensor.matmul(pw[:], lhsT=S[:, t, :], rhs=R[:, t, :],
                                         start=(t == 0), stop=(t == tpw - 1))

                # out = psum[:, :64] / (z + 1e-16) + bias
                zi = epool.tile([128, 8], f32, tag="zi")
                nc.vector.tensor_scalar_add(zi[:], pw[:, 64:72], 1e-16)
                rz = epool.tile([128, 8], f32, tag="rz")
                nc.vector.reciprocal(rz[:], zi[:])
                xm = epool.tile([128, 64], f32, tag="xm")
                nc.vector.tensor_tensor(
                    xm[:].rearrange("p (h c) -> p h c", h=8),
                    pw[:, 0:64].rearrange("p (h c) -> p h c", h=8),
                    rz[:].unsqueeze(2).broadcast_to([128, 8, 8]),
                    Alu.mult)
                nc.vector.tensor_tensor(sb_x[:, w, :], xm[:], sb_brep[:, l, :],
                                        Alu.add)

        nc.sync.dma_start(t_out.ap().rearrange("(t p) c -> p t c", p=128),
                          sb_x[:])

    nc.finalize()
    return nc


def _get_program(tpw):
    if tpw not in _cache:
        _cache[tpw] = _build(tpw)
    return _cache[tpw]


# ----------------------------------------------------------------------------
# Entry point
# ----------------------------------------------------------------------------
def make_program_and_inputs(x, edge_index, Ws, att_src, att_dst, biases):
    x = np.asarray(x, dtype=np.float32)
    Ws = np.asarray(Ws, dtype=np.float32)
    att_src = np.asarray(att_src, dtype=np.float32)
    att_dst = np.asarray(att_dst, dtype=np.float32)
    biases = np.asarray(biases, dtype=np.float32)

    tpw, srcidx, dstidx, dloc = _prep_edges(edge_index)
    nc = _get_program(tpw)

    xpad = np.zeros((NPAD, D), np.float32)
    xpad[:N_NODES] = x

    # A2[cout, l, 0:8] = att_src heads, [.., 8:16] = att_dst heads
    a2 = np.zeros((64, L, 16), np.float32)
    for l in range(L):
        for h in range(H):
            a2[h * C:(h + 1) * C, l, h] = att_src[l, h]
            a2[h * C:(h + 1) * C, l, 8 + h] = att_dst[l, h]
    # wts[cin, l, 0:64] = W; [cin, l, 64:80] = W @ A2  (s = x @ (W A2))
    wts = np.zeros((64, L, 80), np.float32)
    for l in range(L):
        wts[:, l, 0:64] = Ws[l]
        wts[:, l, 64:80] = Ws[l] @ a2[:, l, :]
    brep = np.broadcast_to(biases[None, :, :], (128, L, 64)).copy()
    iota = np.tile(np.arange(128, dtype=ml_dtypes.bfloat16), (128, 1))
    ident = np.eye(128, dtype=np.float32)

    common = dict(wts=wts, brep=brep, iota=iota, ident=ident)
    in_maps = []
    for c in range(NCORES):
        in_maps.append(dict(
            xsh=np.ascontiguousarray(xpad[c * NSH:(c + 1) * NSH]),
            srcidx=srcidx[c], dstidx=dstidx[c],
            dstloc=np.ascontiguousarray(dloc[c]),
            **common))
    return nc, in_maps


def kernel(x, edge_index, Ws, att_src, att_dst, biases):
    from concourse.bass_utils import run_bass_kernel_spmd

    nc, in_maps = make_program_and_inputs(
        x, edge_index, Ws, att_src, att_dst, biases)
    res = run_bass_kernel_spmd(nc, in_maps, core_ids=list(range(NCORES)))
    out = np.concatenate([res.results[c]["out"] for c in range(NCORES)], axis=0)
    return out[:N_NODES]



# revision 8
# speedup vs baseline: 5.3368x; 5.3368x over previous
"""GAT (8-layer, 8-head) Trainium2 Bass kernel v2, 8-core SPMD.

Strategy (degree-partitioned edge layout; gather + reduce, no scatter matmuls):
- Host: add self-loops; sort nodes by in-degree (desc); permuted node blocks
  of 128 are degree-homogeneous. Deal blocks round-robin: block r ->
  core r%8, window r//8; table id of new-node q is
  (r%8)*2560 + (r//8)*128 + q%128.  Per (core, window): partition p =
  dst-local index, tile t = per-dst edge rank.  T[wl] = max degree over the
  8 blocks of window wl (static, shared by all cores).
- Device, per layer: fused [h|s] = x @ [W | W@A2] (bf16 PE), node-major
  256B table rows [h bf16 0:64 | s_src f32@32:40 | s_dst f32@40:48],
  AllGather -> TAB.  Per window: dma_gather rows by src (slot partition =
  dst-local), e = lrelu(s_src + s_dst[p]) where s_dst is a per-partition
  free-dim broadcast of the local table rows, ex = exp(e) written bf16 into
  gathered cols 64:72, R = [ex*h | ex] built in place, one tensor_reduce
  over the tile axis accumulates out[p] and z[p].  No dst gather, no
  one-hot, no PE scatter.
- Padding slots point at a sentinel row with s_src = -1e30 (=> ex = 0).
"""

import numpy as np
import ml_dtypes

N_NODES = 20000
N_EDGES = 640000
L, H, C = 8, 8, 8
D = H * C  # 64
NEG_SLOPE = 0.2

NCORES = 8
WIN = 128
WPC = 20                  # windows per core
NSH = WIN * WPC           # 2560 nodes per shard
NPAD = NCORES * NSH       # 20480
SENT = NPAD               # sentinel table row
TROWS = NPAD + 1

_cache = {}
SIM1 = False              # single-core variant for TimelineSim (fake allgather)


# ----------------------------------------------------------------------------
# Host preprocessing
# ----------------------------------------------------------------------------
def _prep_edges(edge_index):
    src = np.asarray(edge_index[0], dtype=np.int64).astype(np.int32)
    dst = np.asarray(edge_index[1], dtype=np.int64).astype(np.int32)
    loops = np.arange(N_NODES, dtype=np.int32)
    src = np.concatenate([src, loops])
    dst = np.concatenate([dst, loops])

    deg = np.bincount(dst, minlength=N_NODES)          # incl. self loop
    order = np.argsort(-deg, kind="stable")            # orig ids, deg desc
    newid = np.empty(N_NODES, np.int32)
    newid[order] = np.arange(N_NODES, dtype=np.int32)

    q_all = np.arange(NPAD, dtype=np.int32)
    r_all = q_all // 128
    tab_of_q = (r_all % NCORES) * NSH + (r_all // NCORES) * 128 + (q_all % 128)

    degq = deg[order]                                   # deg sorted desc
    T = tuple(int(degq[w * 1024]) for w in range(WPC))
    assert all(t > 0 for t in T)
    off = np.zeros(WPC, np.int64)
    off[1:] = np.cumsum(T[:-1])
    nslot = int(sum(T)) * 128

    qd = newid[dst]
    ts = tab_of_q[newid[src]].astype(np.int16)

    o2 = np.argsort(qd, kind="stable")
    qds = qd[o2]
    newgrp = np.empty(len(qds), bool)
    newgrp[0] = True
    newgrp[1:] = qds[1:] != qds[:-1]
    gidx = np.cumsum(newgrp) - 1
    gstart = np.flatnonzero(newgrp)
    t_rank = np.arange(len(qds), dtype=np.int64) - gstart[gidx]

    core_s = (qds // 128) % NCORES
    wl_s = qds // 1024
    p_s = qds % 128
    pos = (off[wl_s] + t_rank) * 128 + p_s

    srcslot = np.full((NCORES, nslot), SENT, dtype=np.int16)
    srcslot[core_s, pos] = ts[o2]
    # wrap16: flat k -> [k % 16, k // 16]
    srci = np.stack([srcslot[c].reshape(-1, 16).T.copy()
                     for c in range(NCORES)])            # [8, 16, nslot/16]

    return T, srci, order


# ----------------------------------------------------------------------------
# Bass program
# ----------------------------------------------------------------------------
def _build(T):
    import concourse.bass as bass
    import concourse.tile as tile
    import concourse.mybir as mybir
    from concourse import bacc
    from contextlib import ExitStack

    f32 = mybir.dt.float32
    bf16 = mybir.dt.bfloat16
    i16 = mybir.dt.int16
    Alu = mybir.AluOpType
    Act = mybir.ActivationFunctionType
    Ax = mybir.AxisListType

    Tmax = max(T)
    off = [0]
    for t in T[:-1]:
        off.append(off[-1] + t)
    nslot = sum(T) * 128
    n16 = nslot // 16

    nc = bacc.Bacc("TRN2", target_bir_lowering=False, debug=False,
                   num_devices=1 if SIM1 else NCORES)

    t_xsh = nc.dram_tensor("xsh", [NSH, D], bf16, kind="ExternalInput")
    t_srci = nc.dram_tensor("srcidx", [16, n16], i16, kind="ExternalInput")
    t_wts = nc.dram_tensor("wts", [64, L, 80], bf16, kind="ExternalInput")
    t_bias = nc.dram_tensor("bias", [1, L * 64], f32, kind="ExternalInput")
    t_out = nc.dram_tensor("out", [NSH, D], bf16, kind="ExternalOutput")

    with tile.TileContext(nc) as tc, ExitStack() as ctx:
        cpool = ctx.enter_context(tc.tile_pool(name="const", bufs=1))
        wpool = ctx.enter_context(tc.tile_pool(name="work", bufs=2))
        gpool = ctx.enter_context(tc.tile_pool(name="gath", bufs=2))
        epool = ctx.enter_context(tc.tile_pool(name="edge", bufs=2))
        dram = ctx.enter_context(tc.tile_pool(name="dram", bufs=1, space="DRAM"))
        psT = ctx.enter_context(tc.tile_pool(name="psT", bufs=2, space="PSUM"))
        psA = ctx.enter_context(tc.tile_pool(name="psA", bufs=2, space="PSUM"))

        # persistent SBUF
        sb_xb = cpool.tile([128, WPC, D], bf16)
        sb_x = cpool.tile([128, WPC, D], f32)
        sb_srci = cpool.tile([128, n16], i16)
        sb_wts = cpool.tile([64, L, 80], bf16)
        sb_bias = cpool.tile([1, L * 64], f32)
        sb_brep = cpool.tile([128, L * 64], f32)
        sb_ident = cpool.tile([128, 128], f32)

        nc.sync.dma_start(sb_xb[:], t_xsh.ap().rearrange("(t p) c -> p t c", p=128))
        nc.vector.tensor_copy(sb_x[:], sb_xb[:])
        for k in range(8):
            nc.sync.dma_start(sb_srci[16 * k:16 * (k + 1), :], t_srci.ap())
        nc.sync.dma_start(sb_wts[:], t_wts.ap())
        nc.sync.dma_start(sb_bias[:], t_bias.ap())
        nc.gpsimd.partition_broadcast(sb_brep[:], sb_bias[:])
        # identity = affine_select(p - j == 0 ? 1 : 0)
        nc.vector.memset(sb_ident[:], 1.0)
        nc.gpsimd.affine_select(sb_ident[:], sb_ident[:], pattern=[[-1, 128]],
                                compare_op=Alu.is_equal, fill=0.0,
                                base=0, channel_multiplier=1)

        TAB = dram.tile([TROWS, 128], bf16)
        STAGE = dram.tile([NSH, 128], bf16)

        # STAGE bf16 cols 96:128 never produced; zero once (keeps sim finite)
        zjunk = cpool.tile([128, WPC, 32], bf16)
        nc.vector.memset(zjunk[:], 0.0)
        nc.sync.dma_start(
            STAGE[:, 96:128].rearrange("(t p) c -> p t c", p=128), zjunk[:])

        # sentinel row: h=0, s_src=-1e30 => ex = 0 for padding slots
        sent = cpool.tile([1, 128], bf16)
        nc.vector.memset(sent[:], 0.0)
        nc.vector.memset(sent[:].bitcast(f32)[:, 32:40], -1e30)
        nc.sync.dma_start(TAB[SENT:SENT + 1, :], sent[:])

        for l in range(L):
            # ---------------- phase A: projection + table ------------------
            xT = wpool.tile([64, NSH], bf16, tag="xT")
            for t in range(WPC):
                pt = psT.tile([64, 128], f32, tag="psTa")
                nc.tensor.transpose(pt[:], sb_x[:, t, :], sb_ident[:])
                nc.scalar.copy(xT[:, t * 128:(t + 1) * 128], pt[:])

            hsT = wpool.tile([80, NSH], f32, tag="hsT")
            for k0 in range(0, NSH, 512):
                k1 = min(k0 + 512, NSH)
                ph = psA.tile([80, k1 - k0], f32, tag="psA")
                nc.tensor.matmul(ph[:], lhsT=sb_wts[:, l, :],
                                 rhs=xT[:, k0:k1], start=True, stop=True)
                nc.scalar.copy(hsT[:, k0:k1], ph[:])

            tabsb = wpool.tile([128, WPC, 128], bf16, tag="tabsb")
            for t in range(WPC):
                pt = psT.tile([128, 80], f32, tag="psTb")
                nc.tensor.transpose(pt[:], hsT[:, t * 128:(t + 1) * 128],
                                    sb_ident[:80, :80])
                nc.scalar.copy(tabsb[:, t, 0:64], pt[:, 0:64])
                nc.vector.tensor_copy(
                    tabsb[:, t, :].bitcast(f32)[:, 32:48], pt[:, 64:80])

            nc.sync.dma_start(
                STAGE[:, 0:96].rearrange("(t p) c -> p t c", p=128),
                tabsb[:, :, 0:96])
            if SIM1:
                for c in range(NCORES):
                    nc.sync.dma_start(TAB[c * NSH:(c + 1) * NSH, :], STAGE[:])
            else:
                nc.gpsimd.collective_compute(
                    "AllGather", Alu.bypass,
                    replica_groups=[list(range(NCORES))],
                    ins=[STAGE[:].opt()],
                    outs=[TAB[0:NPAD, :].opt()],
                )

            # ---------------- phase B: edges, per window -------------------
            layerbuf = wpool.tile([128, WPC, 72], f32, tag="layerbuf")
            for w in range(WPC):
                Tw = T[w]
                nW = 128 * Tw
                i0 = 128 * off[w]
                vs = gpool.tile([128, Tmax, 128], bf16, tag="vs")
                for j0 in range(0, nW, 1024):
                    j1 = min(j0 + 1024, nW)
                    n = j1 - j0
                    nc.gpsimd.dma_gather(
                        out_ap=vs[:, j0 // 128:j1 // 128, :], in_ap=TAB[:],
                        idxs_ap=sb_srci[:, (i0 + j0) // 16:(i0 + j1) // 16],
                        num_idxs=n, num_idxs_reg=n, elem_size=128)

                e = epool.tile([128, Tmax, 8], f32, tag="e")
                nc.vector.tensor_tensor(
                    e[:, :Tw], vs[:, :Tw, :].bitcast(f32)[:, :, 32:40],
                    tabsb[:, w, :].bitcast(f32)[:, 40:48]
                        .unsqueeze(1).broadcast_to([128, Tw, 8]),
                    Alu.add)
                nc.vector.scalar_tensor_tensor(
                    e[:, :Tw], e[:, :Tw], NEG_SLOPE, e[:, :Tw],
                    op0=Alu.mult, op1=Alu.max)
                # ex -> bf16, written into gathered cols 64:72
                nc.scalar.activation(vs[:, :Tw, 64:72], e[:, :Tw], Act.Exp)
                # R = [h*ex | ex] in place
                nc.vector.tensor_tensor(
                    vs[:, :Tw, 0:64].rearrange("p t (h c) -> p t h c", h=8),
                    vs[:, :Tw, 0:64].rearrange("p t (h c) -> p t h c", h=8),
                    vs[:, :Tw, 64:72].unsqueeze(3).broadcast_to(
                        [128, Tw, 8, 8]),
                    Alu.mult)
                # out[p, 0:72] = sum over t
                nc.vector.tensor_reduce(
                    layerbuf[:, w, :],
                    vs[:, :Tw, 0:72].rearrange("p t c -> p c t"),
                    axis=Ax.X, op=Alu.add)

            # ---------------- finals: x = out/(z+eps) + b ------------------
            zi = epool.tile([128, WPC, 8], f32, tag="zi")
            nc.vector.tensor_scalar_add(zi[:], layerbuf[:, :, 64:72], 1e-16)
            nc.vector.reciprocal(zi[:], zi[:])
            nc.vector.tensor_tensor(
                sb_x[:].rearrange("p w (h c) -> p w h c", h=8),
                layerbuf[:, :, 0:64].rearrange("p w (h c) -> p w h c", h=8),
                zi[:].unsqueeze(3).broadcast_to([128, WPC, 8, 8]),
                Alu.mult)
            nc.vector.tensor_tensor(
                sb_x[:], sb_x[:],
                sb_brep[:, l * 64:(l + 1) * 64].unsqueeze(1)
                    .broadcast_to([128, WPC, 64]),
                Alu.add)

        ob = cpool.tile([128, WPC, D], bf16)
        nc.vector.tensor_copy(ob[:], sb_x[:])
        nc.sync.dma_start(t_out.ap().rearrange("(t p) c -> p t c", p=128),
                          ob[:])

    nc.finalize()
    return nc


def _get_program(T):
    key = (tuple(T), SIM1)
    if key not in _cache:
        _cache[key] = _build(T)
    return _cache[key]


# ----------------------------------------------------------------------------
# Entry point
# ----------------------------------------------------------------------------
def make_program_and_inputs(x, edge_index, Ws, att_src, att_dst, biases):
    x = np.asarray(x, dtype=np.float32)
    Ws = np.asarray(Ws, dtype=np.float32)
    att_src = np.asarray(att_src, dtype=np.float32)
    att_dst = np.asarray(att_dst, dtype=np.float32)
    biases = np.asarray(biases, dtype=np.float32)

    T, srci, order = _prep_edges(edge_index)
    nc = _get_program(T)

    # per-core x shards in (window, pos) order
    m = np.arange(NSH)
    xsh = []
    for c in range(NCORES):
        q = ((m // 128) * NCORES + c) * 128 + (m % 128)
        xc = np.zeros((NSH, D), np.float32)
        real = q < N_NODES
        xc[real] = x[order[q[real]]]
        xsh.append(xc.astype(ml_dtypes.bfloat16))

    a2 = np.zeros((64, L, 16), np.float32)
    for l in range(L):
        for h in range(H):
            a2[h * C:(h + 1) * C, l, h] = att_src[l, h]
            a2[h * C:(h + 1) * C, l, 8 + h] = att_dst[l, h]
    wts = np.zeros((64, L, 80), np.float32)
    for l in range(L):
        wts[:, l, 0:64] = Ws[l]
        wts[:, l, 64:80] = Ws[l] @ a2[:, l, :]
    wts = wts.astype(ml_dtypes.bfloat16)
    bias = biases.reshape(1, L * 64).copy()

    in_maps = [dict(xsh=xsh[c], srcidx=np.ascontiguousarray(srci[c]),
                    wts=wts, bias=bias) for c in range(NCORES)]

    # output reassembly indices: out_full[order[q]] = res[core(q)][m(q)]
    q = np.arange(N_NODES)
    core_q = (q // 128) % NCORES
    m_q = (q // 1024) * 128 + (q % 128)
    return nc, in_maps, (order, core_q, m_q)


def assemble_output(res, meta):
    order, core_q, m_q = meta
    shards = [np.asarray(res.results[c]["out"]).astype(np.float32)
              for c in range(NCORES)]
    allout = np.stack(shards)                          # [8, 2560, 64]
    out = np.empty((N_NODES, D), np.float32)
    out[order] = allout[core_q, m_q]
    return out


def kernel(x, edge_index, Ws, att_src, att_dst, biases):
    from concourse.bass_utils import run_bass_kernel_spmd

    nc, in_maps, meta = make_program_and_inputs(
        x, edge_index, Ws, att_src, att_dst, biases)
    res = run_bass_kernel_spmd(nc, in_maps, core_ids=list(range(NCORES)))
    return assemble_output(res, meta)
